# revision 26
# baseline (speedup 1.0000x reference)
"""AttnBlock (GroupNorm + 4-head d=128 self-attention + residual).

Full input x: [8, 512, 2048] fp32. Data-parallel over batch: core b computes
batch b entirely on-chip (no collectives).

Per-core math (C=512, L=2048, G=4 groups, NH=4 heads, HD=128):
  h  = groupnorm(x)                    (group == one 128-partition tile)
  q  = wq @ h + bq   [d, l] layout
  k  = wk @ h + bk   [d, l]
  vT = h^T @ wv^T + bv  [l, d] layout  (produced transposed; no V transposes)
  sT[k,q] = k_tile^T q  -> exp (no max-sub; logits ~ N(0,1))
  den = ones^T (4-way k-tile group sums built on DVE), matmul'd on PE
  avT[d,q] = sum_kt vT_tile^T e_kt ; attn = avT * (1/den)
  out = wo @ attn + bo + x

Weights arrive PRE-TRANSPOSED from the host wrapper (w^T, [c_in, c_out] row
major), so they DMA directly into the stationary-operand layout — no PE
transposes — on the otherwise-idle GpSimd DMA queue while x keeps the sync
queue. GroupNorm h and the q/k/v weights run as float32r; the attention path
(q, k, vT, exp(s), attn) is bf16, letting one scalar-engine Exp cover a
[128, 1024] score tile (two adjacent PSUM banks).

Attention runs as ONE continuous software pipeline across all 16 (h, qc)
chunks — no per-chunk flush, so chunk boundaries cost nothing. The softmax
denominator uses a DVE add tree (pair sums, then 4-tile group sums) so the
PE sees only 4 ones-matmuls per chunk; 1/den is a single-op approximate
reciprocal (~18 bits) that neither holds the den PSUM bank nor clogs the
DVE queue. The out-projection for q-chunk i is spread one ot-chain per
h-chunk through q-chunk i+1, sharing the den-bank rotation in strict
alternation.

PSUM (8 banks): s4 [128,1024]x2 (4) + av [128,512]x2 (2) + den/op x2 (2).
"""

import os
import numpy as np

import concourse.bass as bass
import concourse.tile as tile
from concourse import bacc, mybir
from concourse.bass_utils import run_bass_kernel_spmd

F32 = mybir.dt.float32
F32R = mybir.dt.float32r
BF16 = mybir.dt.bfloat16

B, C, L = 8, 512, 2048
G = 4            # groupnorm groups; group size 128 == one partition tile
NH, HD = 4, 128  # heads, head dim
CT = C // 128    # 4 channel tiles
LC = L // 512    # 4 l-chunks of 512
LT = L // 128    # 16 l-tiles of 128
NP = LT // 2     # 8 k-tile pairs
EPS = 1e-6
SM_SCALE = float(HD) ** -0.5

AFT = mybir.ActivationFunctionType
ALU = mybir.AluOpType


def build_attn_block(nc):
    x_d = nc.dram_tensor("x", [C, L], F32, kind="ExternalInput").ap()
    gs_d = nc.dram_tensor("gn_scale", [C], F32, kind="ExternalInput").ap()
    gb_d = nc.dram_tensor("gn_bias", [C], F32, kind="ExternalInput").ap()
    w_d = {}
    b_d = {}
    for nm in ("q", "k", "v", "o"):
        # host passes w^T: [c_in, c_out] row-major
        w_d[nm] = nc.dram_tensor(f"w{nm}", [C, C], F32, kind="ExternalInput").ap()
        b_d[nm] = nc.dram_tensor(f"b{nm}", [C], F32, kind="ExternalInput").ap()
    out_d = nc.dram_tensor("out", [C, L], F32, kind="ExternalOutput").ap()

    with tile.TileContext(nc) as tc:
        with (
            tc.tile_pool(name="const", bufs=1) as const,
            tc.tile_pool(name="wstage", bufs=2) as wstage,
            tc.tile_pool(name="wt", bufs=1) as wt,
            tc.tile_pool(name="big", bufs=1) as big,
            tc.tile_pool(name="small", bufs=4) as small,
            tc.tile_pool(name="epool", bufs=5) as epool,
            tc.tile_pool(name="spool", bufs=4) as spool,
            tc.tile_pool(name="cpool", bufs=2) as cpool,
            tc.tile_pool(name="psum", bufs=2, space="PSUM") as psum,
        ):
            # ---- constants ----
            ones = const.tile([128, 128], F32)
            nc.vector.memset(ones, 1.0)
            ones_bf = const.tile([128, 128], BF16)
            nc.vector.memset(ones_bf, 1.0)
            eps_t = const.tile([128, 1], F32)
            nc.vector.memset(eps_t, EPS)

            def load_cvec(name, ap_1d):
                t = const.tile([128, CT], F32, name=name)
                nc.sync.dma_start(out=t, in_=ap_1d.rearrange("(t p) -> p t", p=128))
                return t

            bq_sb = load_cvec("bq_sb", b_d["q"])
            bk_sb = load_cvec("bk_sb", b_d["k"])
            bo_sb = load_cvec("bo_sb", b_d["o"])
            gs_sb = load_cvec("gs_sb", gs_d)
            gb_sb = load_cvec("gb_sb", gb_d)

            bv_bc = const.tile([128, C], F32, name="bv_bc")  # bv broadcast
            nc.sync.dma_start(
                out=bv_bc,
                in_=bass.AP(
                    tensor=b_d["v"].tensor,
                    offset=b_d["v"].offset,
                    ap=[[0, 128]] + list(b_d["v"].ap),
                ),
            )

            # ---- x: load fully resident [p, ct, l] (sync queue);
            #      quarter-sized DMAs so bn_stats starts on early data ----
            x_r = x_d.rearrange("(t p) l -> p t l", p=128)
            x_sb = big.tile([128, CT, L], F32, tag="x_sb")
            for ct in range(CT):
                for i in range(4):
                    eng = nc.sync if (ct * 4 + i) % 2 == 0 else nc.scalar
                    eng.dma_start(
                        out=x_sb[:, ct, i * 512 : (i + 1) * 512],
                        in_=x_r[:, ct, i * 512 : (i + 1) * 512],
                    )

            # ---- HAM warm-up: matmuls over the arriving x quarters (results
            #      unused) keep the PE active through the DMA window so the
            #      real work starts at the warm 2.4 GHz clock ----
            for ct in range(2, CT):
                for i in range(4):
                    wp = psum.tile([128, 512], F32, tag="av")
                    nc.tensor.matmul(
                        wp, ones, x_sb[:, ct, i * 512 : (i + 1) * 512],
                        start=True, stop=True,
                    )

            # ---- weights: DMA host-pretransposed w^T (gpsimd queue, overlaps
            #      the x loads), convert fp32 -> fp32r / bf16 (o) ----
            wts = {}
            for nm in ("q", "k", "v"):
                wts[nm] = wt.tile([128, CT, C], F32R, name=f"w{nm}t")
            wts["o"] = wt.tile([128, CT, C], BF16, name="wot")
            w_r = {nm: w_d[nm].rearrange("(t p) o -> p t o", p=128) for nm in w_d}
            w_stgs = {}
            for nm in ("v", "q", "k", "o"):
                # shared 2-slot staging: the k/o loads wait (on the idle
                # gpsimd queue) until the v/q slots are converted
                stg = wstage.tile([128, CT, C], F32, tag="stg", bufs=2)
                nc.gpsimd.dma_start(out=stg, in_=w_r[nm])
                w_stgs[nm] = stg

            def convert_weight(nm, engine):
                stg = w_stgs[nm]
                for ct in range(CT):
                    if engine == "scalar":
                        nc.scalar.copy(wts[nm][:, ct, :], stg[:, ct, :])
                    else:
                        nc.vector.tensor_copy(wts[nm][:, ct, :], stg[:, ct, :])

            # wv converts early on the still-idle ACT (V proj needs it first)
            convert_weight("v", "scalar")

            # ---- groupnorm stats, per channel tile ----
            h_sb = big.tile([128, CT, L], F32R, tag="h_sb")
            gn_ab = []
            for ct in range(CT):
                stats = small.tile([128, 4, 6], F32, tag="stats")
                for i in range(4):
                    nc.vector.bn_stats(
                        out=stats[:, i, :], in_=x_sb[:, ct, i * 512 : (i + 1) * 512]
                    )
                mv = small.tile([128, 2], F32, tag="mv")
                nc.vector.bn_aggr(out=mv, in_=stats)
                # stat2 = [mean_p, E[x^2]_p]
                stat2 = small.tile([128, 2], F32, tag="stat2")
                nc.vector.tensor_copy(stat2[:, 0:1], mv[:, 0:1])
                nc.vector.scalar_tensor_tensor(
                    out=stat2[:, 1:2],
                    in0=mv[:, 0:1],
                    scalar=mv[:, 0:1],
                    in1=mv[:, 1:2],
                    op0=ALU.mult,
                    op1=ALU.add,
                )
                pg = psum.tile([128, 2], F32, tag="den")
                nc.tensor.matmul(pg, ones, stat2, start=True, stop=True)
                mean_t = small.tile([128, 1], F32, tag="mean_t")
                nc.vector.tensor_scalar_mul(mean_t, pg[:, 0:1], 1.0 / 128.0)
                ex2_t = small.tile([128, 1], F32, tag="ex2_t")
                nc.vector.tensor_scalar_mul(ex2_t, pg[:, 1:2], 1.0 / 128.0)
                var_t = small.tile([128, 1], F32, tag="var_t")
                nc.vector.tensor_mul(var_t, mean_t, mean_t)
                nc.vector.tensor_sub(var_t, ex2_t, var_t)
                std_t = small.tile([128, 1], F32, tag="std_t")
                nc.scalar.activation(std_t, var_t, AFT.Sqrt, bias=eps_t)
                rstd_t = small.tile([128, 1], F32, tag="rstd_t")
                nc.vector.reciprocal_approx_fast(rstd_t, std_t)
                a_t = small.tile([128, 1], F32, tag="a_t", bufs=CT)
                nc.vector.tensor_mul(a_t, rstd_t, gs_sb[:, ct : ct + 1])
                b_t = small.tile([128, 1], F32, tag="b_t", bufs=CT)
                nc.vector.tensor_mul(b_t, mean_t, a_t)
                nc.vector.tensor_sub(b_t, gb_sb[:, ct : ct + 1], b_t)
                gn_ab.append((a_t, b_t))
                for l2 in range(2):
                    if ct % 2 == 0:
                        nc.scalar.activation(
                            h_sb[:, ct, l2 * 1024 : (l2 + 1) * 1024],
                            x_sb[:, ct, l2 * 1024 : (l2 + 1) * 1024],
                            AFT.Identity,
                            bias=b_t,
                            scale=a_t,
                        )
                    else:
                        nc.vector.tensor_scalar(
                            out=h_sb[:, ct, l2 * 1024 : (l2 + 1) * 1024],
                            in0=x_sb[:, ct, l2 * 1024 : (l2 + 1) * 1024],
                            scalar1=a_t,
                            scalar2=b_t,
                            op0=ALU.mult,
                            op1=ALU.add,
                        )

            # q/k weights convert on ACT behind the applies; wo on DVE
            convert_weight("q", "scalar")
            convert_weight("k", "scalar")
            convert_weight("o", "vector")

            # ---- projections: V (av-tag PSUM) and q/k (s4-tag PSUM) chains
            #      interleaved so the PE stream is dense while ACT drains the
            #      q/k bias copies ----
            vT_sb = big.tile([128, LT, C], BF16, tag="vT_sb")
            q_sb = big.tile([128, NH, L], BF16, tag="q_sb")
            k_sb = big.tile([128, NH, L], BF16, tag="k_sb")

            def emit_v_chain(lt):
                pp = psum.tile([128, 512], F32, tag="av")
                for ct in range(CT):
                    nc.tensor.matmul(
                        pp,
                        h_sb[:, ct, lt * 128 : (lt + 1) * 128],
                        wts["v"][:, ct, :],
                        start=(ct == 0),
                        stop=(ct == CT - 1),
                    )
                nc.vector.tensor_add(vT_sb[:, lt, :], pp, bv_bc)

            def emit_qk_chain(h, which, l2):
                dst, wtt, bias = (
                    (k_sb, wts["k"], bk_sb) if which == "k" else (q_sb, wts["q"], bq_sb)
                )
                for lc2 in range(2):
                    pp = psum.tile([128, 512], F32, tag="s4")
                    for ct in range(CT):
                        nc.tensor.matmul(
                            pp,
                            wtt[:, ct, h * 128 : (h + 1) * 128],
                            h_sb[
                                :, ct, (l2 * 2 + lc2) * 512 : (l2 * 2 + lc2 + 1) * 512
                            ],
                            start=(ct == 0),
                            stop=(ct == CT - 1),
                        )
                    lc = l2 * 2 + lc2
                    nc.scalar.activation(
                        dst[:, h, lc * 512 : (lc + 1) * 512],
                        pp,
                        AFT.Identity,
                        bias=bias[:, h : h + 1],
                    )

            qk_chains = [
                (h, which, l2)
                for h in range(NH)
                for which in ("k", "q")
                for l2 in range(2)
            ]
            # V lt0..7 need only the first l-half of h; then alternate the
            # remaining V chains with the 16 q/k chains
            for lt in range(4):
                emit_v_chain(lt)
            ci = 0
            for lt in range(4, 16, 2):
                emit_qk_chain(*qk_chains[ci]); ci += 1
                emit_v_chain(lt)
                emit_v_chain(lt + 1)
            while ci < len(qk_chains):
                emit_qk_chain(*qk_chains[ci]); ci += 1

            # ---- attention: one continuous pipeline over all (qc, h) ----
            attn_sb = big.tile([128, NH, L], BF16, tag="attn_sb")

            def emit_qk_exp(h, qc, pr):
                ps = psum.tile([128, 1024], F32, tag="s4", bufs=2)
                for j in range(2):
                    kt = 2 * pr + j
                    nc.tensor.matmul(
                        ps[:, j * 512 : (j + 1) * 512],
                        k_sb[:, h, kt * 128 : (kt + 1) * 128],
                        q_sb[:, h, qc * 512 : (qc + 1) * 512],
                        start=True,
                        stop=True,
                    )
                e2 = epool.tile([128, 1024], BF16, tag="e2", bufs=5)
                nc.scalar.activation(e2, ps, AFT.Exp, scale=SM_SCALE)
                return e2

            esum_live = []  # esum tiles of the current 4-k-tile group

            def emit_den_av(h, qc, pr, e2, pden, pav):
                esum = spool.tile([128, 512], BF16, tag="esum", bufs=4)
                nc.vector.tensor_add(esum, e2[:, 0:512], e2[:, 512:1024])
                esum_live.append(esum)
                if pr % 2 == 1:
                    esum2 = spool.tile([128, 512], BF16, tag="esum2", bufs=3)
                    nc.vector.tensor_add(esum2, esum_live[0], esum_live[1])
                    esum_live.clear()
                    nc.tensor.matmul(
                        pden, ones_bf, esum2, start=(pr == 1), stop=(pr == NP - 1)
                    )
                for j in range(2):
                    kt = 2 * pr + j
                    nc.tensor.matmul(
                        pav,
                        vT_sb[:, kt, h * 128 : (h + 1) * 128],
                        e2[:, j * 512 : (j + 1) * 512],
                        start=(pr == 0 and j == 0),
                        stop=(pr == NP - 1 and j == 1),
                    )

            def finish_chunk(h, qc, pden, pav):
                # single-op approximate reciprocal (~18 bits, plenty for a
                # softmax denominator): fast enough to neither hold the den
                # bank nor clog the DVE queue
                rden = cpool.tile([128, 512], F32, tag="rden", bufs=2, name="rden")
                nc.vector.reciprocal_approx_fast(rden, pden)
                nc.vector.tensor_mul(
                    attn_sb[:, h, qc * 512 : (qc + 1) * 512], pav, rden
                )

            def emit_out_proj_ot(lc, ot):
                pp = psum.tile([128, 512], F32, tag="den", name="pp")
                for ct in range(CT):
                    nc.tensor.matmul(
                        pp,
                        wts["o"][:, ct, ot * 128 : (ot + 1) * 128],
                        attn_sb[:, ct, lc * 512 : (lc + 1) * 512],
                        start=(ct == 0),
                        stop=(ct == CT - 1),
                    )
                ot_sb = cpool.tile([128, 512], F32, tag="ot_sb")
                nc.vector.scalar_tensor_tensor(
                    out=ot_sb,
                    in0=pp,
                    scalar=bo_sb[:, ot : ot + 1],
                    in1=x_sb[:, ot, lc * 512 : (lc + 1) * 512],
                    op0=ALU.add,
                    op1=ALU.add,
                )
                nc.sync.dma_start(
                    out=out_d[ot * 128 : (ot + 1) * 128, lc * 512 : (lc + 1) * 512],
                    in_=ot_sb,
                )

            DEPTH = 3  # den/av lag QK+exp by this many k-tile pairs

            def drain_one(pq):
                p = pq.pop(0)
                emit_den_av(*p)
                if p[2] == NP - 1:
                    finish_chunk(p[0], p[1], p[4], p[5])

            pipeline = []
            deferred_out = None  # l-chunk whose out-projection awaits emission
            for qc in range(LC):
                for h in range(NH):
                    pden = psum.tile([128, 512], F32, tag="den")
                    pav = psum.tile([128, 512], F32, tag="av")
                    for pr in range(NP):
                        e2 = emit_qk_exp(h, qc, pr)
                        if len(pipeline) >= DEPTH:
                            drain_one(pipeline)
                        pipeline.append((h, qc, pr, e2, pden, pav))
                        # the previous q-chunk's out-projection: one ot-chain
                        # per h-chunk, keeping the den-bank rotation strictly
                        # alternating den(h) / op(ot)
                        if deferred_out is not None and pr == 4:
                            emit_out_proj_ot(deferred_out, h)
                deferred_out = qc
            while pipeline:
                drain_one(pipeline)
            for ot in range(CT):
                emit_out_proj_ot(deferred_out, ot)
    nc.compile()
    return nc


_NC_CACHE = {}


def _get_nc():
    if "nc" not in _NC_CACHE:
        nc = bacc.Bacc("TRN2", debug=False)
        build_attn_block(nc)
        _NC_CACHE["nc"] = nc
    return _NC_CACHE["nc"]


def run(trace=False, **inputs):
    nc = _get_nc()
    xs = np.ascontiguousarray(np.asarray(inputs["x"], dtype=np.float32))
    shared = {}
    for nm in ("gn_scale", "gn_bias", "bq", "bk", "bv", "bo"):
        shared[nm] = np.ascontiguousarray(np.asarray(inputs[nm], dtype=np.float32))
    for nm in ("wq", "wk", "wv", "wo"):
        # device kernel consumes pre-transposed weights (stationary layout)
        shared[nm] = np.ascontiguousarray(
            np.asarray(inputs[nm], dtype=np.float32).T
        )
    in_maps = [dict(shared, x=xs[b]) for b in range(B)]
    res = run_bass_kernel_spmd(nc, in_maps, core_ids=list(range(B)), trace=trace)
    out = np.stack([res.results[b]["out"] for b in range(B)], axis=0)
    return out, res


def kernel(**inputs):
    out, _ = run(trace=bool(os.environ.get("ATTN_TRACE")), **inputs)
    return out


# revision 27
# speedup vs baseline: 1.0179x; 1.0179x over previous
"""AttnBlock (GroupNorm + 4-head d=128 self-attention + residual).

Full input x: [8, 512, 2048] fp32. Data-parallel over batch: core b computes
batch b entirely on-chip (no collectives).

Per-core math (C=512, L=2048, G=4 groups, NH=4 heads, HD=128):
  h  = groupnorm(x)                    (group == one 128-partition tile)
  q  = wq @ h + bq   [d, l] layout
  k  = wk @ h + bk   [d, l]
  vT = h^T @ wv^T + bv  [l, d] layout  (produced transposed; no V transposes)
  sT[k,q] = k_tile^T q  -> exp (no max-sub; logits ~ N(0,1))
  den = ones^T (4-way k-tile group sums built on DVE), matmul'd on PE
  avT[d,q] = sum_kt vT_tile^T e_kt ; attn = avT * (1/den)
  out = wo @ attn + bo + x

Weights arrive PRE-TRANSPOSED from the host wrapper (w^T, [c_in, c_out] row
major), so they DMA directly into the stationary-operand layout — no PE
transposes — on the otherwise-idle GpSimd DMA queue while x keeps the sync
queue. GroupNorm h and the q/k/v weights run as float32r; the attention path
(q, k, vT, exp(s), attn) is bf16, letting one scalar-engine Exp cover a
[128, 1024] score tile (two adjacent PSUM banks).

Attention runs as ONE continuous software pipeline across all 16 (h, qc)
chunks — no per-chunk flush, so chunk boundaries cost nothing. The softmax
denominator uses a DVE add tree (pair sums, then 4-tile group sums) so the
PE sees only 4 ones-matmuls per chunk; 1/den is a single-op approximate
reciprocal (~18 bits) that neither holds the den PSUM bank nor clogs the
DVE queue. The out-projection for q-chunk i is spread one ot-chain per
h-chunk through q-chunk i+1, sharing the den-bank rotation in strict
alternation.

PSUM (8 banks): s4 [128,1024]x2 (4) + av [128,512]x2 (2) + den/op x2 (2).
"""

import os
import numpy as np

import concourse.bass as bass
import concourse.tile as tile
from concourse import bacc, mybir
from concourse.bass_utils import run_bass_kernel_spmd

F32 = mybir.dt.float32
F32R = mybir.dt.float32r
BF16 = mybir.dt.bfloat16

B, C, L = 8, 512, 2048
G = 4            # groupnorm groups; group size 128 == one partition tile
NH, HD = 4, 128  # heads, head dim
CT = C // 128    # 4 channel tiles
LC = L // 512    # 4 l-chunks of 512
LT = L // 128    # 16 l-tiles of 128
NP = LT // 2     # 8 k-tile pairs
EPS = 1e-6
SM_SCALE = float(HD) ** -0.5

AFT = mybir.ActivationFunctionType
ALU = mybir.AluOpType


def build_attn_block(nc):
    x_d = nc.dram_tensor("x", [C, L], F32, kind="ExternalInput").ap()
    gs_d = nc.dram_tensor("gn_scale", [C], F32, kind="ExternalInput").ap()
    gb_d = nc.dram_tensor("gn_bias", [C], F32, kind="ExternalInput").ap()
    w_d = {}
    b_d = {}
    for nm in ("q", "k", "v", "o"):
        # host passes w^T: [c_in, c_out] row-major
        w_d[nm] = nc.dram_tensor(f"w{nm}", [C, C], F32, kind="ExternalInput").ap()
        b_d[nm] = nc.dram_tensor(f"b{nm}", [C], F32, kind="ExternalInput").ap()
    out_d = nc.dram_tensor("out", [C, L], F32, kind="ExternalOutput").ap()

    with tile.TileContext(nc) as tc:
        with (
            tc.tile_pool(name="const", bufs=1) as const,
            tc.tile_pool(name="wstage", bufs=2) as wstage,
            tc.tile_pool(name="wt", bufs=1) as wt,
            tc.tile_pool(name="big", bufs=1) as big,
            tc.tile_pool(name="small", bufs=4) as small,
            tc.tile_pool(name="epool", bufs=5) as epool,
            tc.tile_pool(name="spool", bufs=4) as spool,
            tc.tile_pool(name="cpool", bufs=2) as cpool,
            tc.tile_pool(name="psum", bufs=2, space="PSUM") as psum,
        ):
            # ---- constants ----
            ones = const.tile([128, 128], F32)
            nc.vector.memset(ones, 1.0)
            ones_bf = const.tile([128, 128], BF16)
            nc.vector.memset(ones_bf, 1.0)
            eps_t = const.tile([128, 1], F32)
            nc.vector.memset(eps_t, EPS)

            def load_cvec(name, ap_1d):
                t = const.tile([128, CT], F32, name=name)
                nc.sync.dma_start(out=t, in_=ap_1d.rearrange("(t p) -> p t", p=128))
                return t

            bq_sb = load_cvec("bq_sb", b_d["q"])
            bk_sb = load_cvec("bk_sb", b_d["k"])
            bo_sb = load_cvec("bo_sb", b_d["o"])
            gs_sb = load_cvec("gs_sb", gs_d)
            gb_sb = load_cvec("gb_sb", gb_d)

            bv_bc = const.tile([128, C], F32, name="bv_bc")  # bv broadcast
            nc.sync.dma_start(
                out=bv_bc,
                in_=bass.AP(
                    tensor=b_d["v"].tensor,
                    offset=b_d["v"].offset,
                    ap=[[0, 128]] + list(b_d["v"].ap),
                ),
            )

            # ---- x: load fully resident [p, ct, l] (sync queue);
            #      quarter-sized DMAs so bn_stats starts on early data ----
            x_r = x_d.rearrange("(t p) l -> p t l", p=128)
            x_sb = big.tile([128, CT, L], F32, tag="x_sb")
            for ct in range(CT):
                for i in range(4):
                    nc.sync.dma_start(
                        out=x_sb[:, ct, i * 512 : (i + 1) * 512],
                        in_=x_r[:, ct, i * 512 : (i + 1) * 512],
                    )

            # ---- HAM warm-up: matmuls over the arriving x quarters (results
            #      unused) keep the PE active through the DMA window so the
            #      real work starts at the warm 2.4 GHz clock ----
            for ct in range(2, CT):
                for i in range(4):
                    wp = psum.tile([128, 512], F32, tag="av")
                    nc.tensor.matmul(
                        wp, ones, x_sb[:, ct, i * 512 : (i + 1) * 512],
                        start=True, stop=True,
                    )

            # ---- weights: DMA host-pretransposed w^T (gpsimd queue, overlaps
            #      the x loads), convert fp32 -> fp32r / bf16 (o) ----
            wts = {}
            for nm in ("q", "k", "v"):
                wts[nm] = wt.tile([128, CT, C], F32R, name=f"w{nm}t")
            wts["o"] = wt.tile([128, CT, C], BF16, name="wot")
            w_r = {nm: w_d[nm].rearrange("(t p) o -> p t o", p=128) for nm in w_d}
            w_stgs = {}
            for nm in ("v", "q", "k", "o"):
                # shared 2-slot staging: the k/o loads wait (on the idle
                # gpsimd queue) until the v/q slots are converted
                stg = wstage.tile([128, CT, C], F32, tag="stg", bufs=2)
                nc.gpsimd.dma_start(out=stg, in_=w_r[nm])
                w_stgs[nm] = stg

            def convert_weight(nm, engine):
                stg = w_stgs[nm]
                for ct in range(CT):
                    if engine == "scalar":
                        nc.scalar.copy(wts[nm][:, ct, :], stg[:, ct, :])
                    else:
                        nc.vector.tensor_copy(wts[nm][:, ct, :], stg[:, ct, :])

            # wv converts early on the still-idle ACT (V proj needs it first)
            convert_weight("v", "scalar")

            # ---- groupnorm stats, per channel tile ----
            h_sb = big.tile([128, CT, L], F32R, tag="h_sb")
            gn_ab = []
            for ct in range(CT):
                stats = small.tile([128, 4, 6], F32, tag="stats")
                for i in range(4):
                    nc.vector.bn_stats(
                        out=stats[:, i, :], in_=x_sb[:, ct, i * 512 : (i + 1) * 512]
                    )
                mv = small.tile([128, 2], F32, tag="mv")
                nc.vector.bn_aggr(out=mv, in_=stats)
                # stat2 = [mean_p, E[x^2]_p]
                stat2 = small.tile([128, 2], F32, tag="stat2")
                nc.vector.tensor_copy(stat2[:, 0:1], mv[:, 0:1])
                nc.vector.scalar_tensor_tensor(
                    out=stat2[:, 1:2],
                    in0=mv[:, 0:1],
                    scalar=mv[:, 0:1],
                    in1=mv[:, 1:2],
                    op0=ALU.mult,
                    op1=ALU.add,
                )
                pg = psum.tile([128, 2], F32, tag="den")
                nc.tensor.matmul(pg, ones, stat2, start=True, stop=True)
                mean_t = small.tile([128, 1], F32, tag="mean_t")
                nc.vector.tensor_scalar_mul(mean_t, pg[:, 0:1], 1.0 / 128.0)
                ex2_t = small.tile([128, 1], F32, tag="ex2_t")
                nc.vector.tensor_scalar_mul(ex2_t, pg[:, 1:2], 1.0 / 128.0)
                var_t = small.tile([128, 1], F32, tag="var_t")
                nc.vector.tensor_mul(var_t, mean_t, mean_t)
                nc.vector.tensor_sub(var_t, ex2_t, var_t)
                std_t = small.tile([128, 1], F32, tag="std_t")
                nc.scalar.activation(std_t, var_t, AFT.Sqrt, bias=eps_t)
                rstd_t = small.tile([128, 1], F32, tag="rstd_t")
                nc.vector.reciprocal_approx_fast(rstd_t, std_t)
                a_t = small.tile([128, 1], F32, tag="a_t", bufs=CT)
                nc.vector.tensor_mul(a_t, rstd_t, gs_sb[:, ct : ct + 1])
                b_t = small.tile([128, 1], F32, tag="b_t", bufs=CT)
                nc.vector.tensor_mul(b_t, mean_t, a_t)
                nc.vector.tensor_sub(b_t, gb_sb[:, ct : ct + 1], b_t)
                gn_ab.append((a_t, b_t))

            # ---- groupnorm apply on ACT (l-half outer: the first half of h
            #      completes early so the V projection can start) ----
            for l2 in range(2):
                for ct in range(CT):
                    a_t, b_t = gn_ab[ct]
                    if ct % 2 == 0:
                        nc.scalar.activation(
                            h_sb[:, ct, l2 * 1024 : (l2 + 1) * 1024],
                            x_sb[:, ct, l2 * 1024 : (l2 + 1) * 1024],
                            AFT.Identity,
                            bias=b_t,
                            scale=a_t,
                        )
                    else:
                        nc.vector.tensor_scalar(
                            out=h_sb[:, ct, l2 * 1024 : (l2 + 1) * 1024],
                            in0=x_sb[:, ct, l2 * 1024 : (l2 + 1) * 1024],
                            scalar1=a_t,
                            scalar2=b_t,
                            op0=ALU.mult,
                            op1=ALU.add,
                        )

            # q/k weights convert on ACT behind the applies; wo on DVE
            convert_weight("q", "scalar")
            convert_weight("k", "scalar")
            convert_weight("o", "vector")

            # ---- projections: V (av-tag PSUM) and q/k (s4-tag PSUM) chains
            #      interleaved so the PE stream is dense while ACT drains the
            #      q/k bias copies ----
            vT_sb = big.tile([128, LT, C], BF16, tag="vT_sb")
            q_sb = big.tile([128, NH, L], BF16, tag="q_sb")
            k_sb = big.tile([128, NH, L], BF16, tag="k_sb")

            def emit_v_chain(lt):
                pp = psum.tile([128, 512], F32, tag="av")
                for ct in range(CT):
                    nc.tensor.matmul(
                        pp,
                        h_sb[:, ct, lt * 128 : (lt + 1) * 128],
                        wts["v"][:, ct, :],
                        start=(ct == 0),
                        stop=(ct == CT - 1),
                    )
                nc.vector.tensor_add(vT_sb[:, lt, :], pp, bv_bc)

            def emit_qk_chain(h, which, l2):
                dst, wtt, bias = (
                    (k_sb, wts["k"], bk_sb) if which == "k" else (q_sb, wts["q"], bq_sb)
                )
                for lc2 in range(2):
                    pp = psum.tile([128, 512], F32, tag="s4")
                    for ct in range(CT):
                        nc.tensor.matmul(
                            pp,
                            wtt[:, ct, h * 128 : (h + 1) * 128],
                            h_sb[
                                :, ct, (l2 * 2 + lc2) * 512 : (l2 * 2 + lc2 + 1) * 512
                            ],
                            start=(ct == 0),
                            stop=(ct == CT - 1),
                        )
                    lc = l2 * 2 + lc2
                    nc.scalar.activation(
                        dst[:, h, lc * 512 : (lc + 1) * 512],
                        pp,
                        AFT.Identity,
                        bias=bias[:, h : h + 1],
                    )

            qk_chains = [
                (h, which, l2)
                for h in range(NH)
                for which in ("k", "q")
                for l2 in range(2)
            ]
            # V lt0..7 need only the first l-half of h; then alternate the
            # remaining V chains with the 16 q/k chains
            for lt in range(4):
                emit_v_chain(lt)
            ci = 0
            for lt in range(4, 16, 2):
                emit_qk_chain(*qk_chains[ci]); ci += 1
                emit_v_chain(lt)
                emit_v_chain(lt + 1)
            while ci < len(qk_chains):
                emit_qk_chain(*qk_chains[ci]); ci += 1

            # ---- attention: one continuous pipeline over all (qc, h) ----
            attn_sb = big.tile([128, NH, L], BF16, tag="attn_sb")

            def emit_qk_exp(h, qc, pr):
                ps = psum.tile([128, 1024], F32, tag="s4", bufs=2)
                for j in range(2):
                    kt = 2 * pr + j
                    nc.tensor.matmul(
                        ps[:, j * 512 : (j + 1) * 512],
                        k_sb[:, h, kt * 128 : (kt + 1) * 128],
                        q_sb[:, h, qc * 512 : (qc + 1) * 512],
                        start=True,
                        stop=True,
                    )
                e2 = epool.tile([128, 1024], BF16, tag="e2", bufs=5)
                nc.scalar.activation(e2, ps, AFT.Exp, scale=SM_SCALE)
                return e2

            esum_live = []  # esum tiles of the current 4-k-tile group

            def emit_den_av(h, qc, pr, e2, pden, pav):
                esum = spool.tile([128, 512], BF16, tag="esum", bufs=4)
                nc.vector.tensor_add(esum, e2[:, 0:512], e2[:, 512:1024])
                esum_live.append(esum)
                if pr % 2 == 1:
                    esum2 = spool.tile([128, 512], BF16, tag="esum2", bufs=3)
                    nc.vector.tensor_add(esum2, esum_live[0], esum_live[1])
                    esum_live.clear()
                    nc.tensor.matmul(
                        pden, ones_bf, esum2, start=(pr == 1), stop=(pr == NP - 1)
                    )
                for j in range(2):
                    kt = 2 * pr + j
                    nc.tensor.matmul(
                        pav,
                        vT_sb[:, kt, h * 128 : (h + 1) * 128],
                        e2[:, j * 512 : (j + 1) * 512],
                        start=(pr == 0 and j == 0),
                        stop=(pr == NP - 1 and j == 1),
                    )

            def finish_chunk(h, qc, pden, pav):
                # single-op approximate reciprocal (~18 bits, plenty for a
                # softmax denominator): fast enough to neither hold the den
                # bank nor clog the DVE queue
                rden = cpool.tile([128, 512], F32, tag="rden", bufs=2, name="rden")
                nc.vector.reciprocal_approx_fast(rden, pden)
                nc.vector.tensor_mul(
                    attn_sb[:, h, qc * 512 : (qc + 1) * 512], pav, rden
                )

            def emit_out_proj_ot(lc, ot):
                pp = psum.tile([128, 512], F32, tag="den", name="pp")
                for ct in range(CT):
                    nc.tensor.matmul(
                        pp,
                        wts["o"][:, ct, ot * 128 : (ot + 1) * 128],
                        attn_sb[:, ct, lc * 512 : (lc + 1) * 512],
                        start=(ct == 0),
                        stop=(ct == CT - 1),
                    )
                ot_sb = cpool.tile([128, 512], F32, tag="ot_sb")
                nc.vector.scalar_tensor_tensor(
                    out=ot_sb,
                    in0=pp,
                    scalar=bo_sb[:, ot : ot + 1],
                    in1=x_sb[:, ot, lc * 512 : (lc + 1) * 512],
                    op0=ALU.add,
                    op1=ALU.add,
                )
                nc.sync.dma_start(
                    out=out_d[ot * 128 : (ot + 1) * 128, lc * 512 : (lc + 1) * 512],
                    in_=ot_sb,
                )

            DEPTH = 3  # den/av lag QK+exp by this many k-tile pairs

            def drain_one(pq):
                p = pq.pop(0)
                emit_den_av(*p)
                if p[2] == NP - 1:
                    finish_chunk(p[0], p[1], p[4], p[5])

            pipeline = []
            deferred_out = None  # l-chunk whose out-projection awaits emission
            for qc in range(LC):
                for h in range(NH):
                    pden = psum.tile([128, 512], F32, tag="den")
                    pav = psum.tile([128, 512], F32, tag="av")
                    for pr in range(NP):
                        e2 = emit_qk_exp(h, qc, pr)
                        if len(pipeline) >= DEPTH:
                            drain_one(pipeline)
                        pipeline.append((h, qc, pr, e2, pden, pav))
                        # the previous q-chunk's out-projection: one ot-chain
                        # per h-chunk, keeping the den-bank rotation strictly
                        # alternating den(h) / op(ot)
                        if deferred_out is not None and pr == 4:
                            emit_out_proj_ot(deferred_out, h)
                deferred_out = qc
            while pipeline:
                drain_one(pipeline)
            for ot in range(CT):
                emit_out_proj_ot(deferred_out, ot)
    nc.compile()
    return nc


_NC_CACHE = {}


def _get_nc():
    if "nc" not in _NC_CACHE:
        nc = bacc.Bacc("TRN2", debug=False)
        build_attn_block(nc)
        _NC_CACHE["nc"] = nc
    return _NC_CACHE["nc"]


def run(trace=False, **inputs):
    nc = _get_nc()
    xs = np.ascontiguousarray(np.asarray(inputs["x"], dtype=np.float32))
    shared = {}
    for nm in ("gn_scale", "gn_bias", "bq", "bk", "bv", "bo"):
        shared[nm] = np.ascontiguousarray(np.asarray(inputs[nm], dtype=np.float32))
    for nm in ("wq", "wk", "wv", "wo"):
        # device kernel consumes pre-transposed weights (stationary layout)
        shared[nm] = np.ascontiguousarray(
            np.asarray(inputs[nm], dtype=np.float32).T
        )
    in_maps = [dict(shared, x=xs[b]) for b in range(B)]
    res = run_bass_kernel_spmd(nc, in_maps, core_ids=list(range(B)), trace=trace)
    out = np.stack([res.results[b]["out"] for b in range(B)], axis=0)
    return out, res


def kernel(**inputs):
    out, _ = run(trace=bool(os.environ.get("ATTN_TRACE")), **inputs)
    return out
